# revision 1
# baseline (speedup 1.0000x reference)
"""Trainium2 Bass kernel for nn_ExcitationModule (YIN pitch -> harmonic synthesis).

Sharding: B=4 rows x 2 halves of T=131072 across 8 cores (pure data parallel;
the phase cumsum carry for the second half of each row is recomputed locally
from the first-half pitch_mult, so no collectives are needed).

Per core layout: 65536 samples as [128 partitions x 512], t = p*512 + q.
Pipeline per core:
  1. YIN on the full row (128 frames on partitions): autocorrelation via a
     2048-point DFT as bf16 PE matmuls (exact linear autocorr since
     1260+630 <= 2048), difference function, CMNDF, threshold/argmax logic.
  2. phase = cumsum(2*pi*f0/FS) via per-partition scan + PE lower-triangular
     prefix matmul + carry.
  3. signal = sum_h amp_h*mask*sin(h*phase): GPSIMD computes h*phase into a
     reversed-segment layout (151 slots: pad + h=150..1), ACT Sin evaluates
     all 150 harmonics per sample, and one DVE tensor_tensor_scan performs
     the masked amp-weighted reduction: state=(sin+state)*data1 with
     data1 = (harm < pi/theta) * telescoping amp ratios, whose running
     product rebuilds amp_h exactly where the Nyquist mask is 1; the pad
     slot (data1=0) resets state, and the cutoff sum is read at the fixed
     segment-end slot -- no per-partition gather (HW has none). The
     reference's +1e-7 mask epsilon term is dropped: it is exactly zero
     whenever sin is zero and otherwise contributes <~1e-6 relative.
"""

import numpy as np
import ml_dtypes
from contextlib import ExitStack

FS = 44100.0
NH = 150
TAU_MIN = 110
FRAME = 1260
B, T = 4, 131072
NF = 128          # frames per row (T//1024)
NFFT = 2048
NBINS = 1024      # DFT bins handled by the main matmul; Nyquist separate
HALF = 65536      # samples per core
P, Q = 128, 512   # per-core layout [P partitions, Q]
JC = 16           # q columns per synthesis chunk
NCHUNK = Q // JC  # 16
SEG = NH + 1      # segment length in scan layout (pad slot + 150 harmonics)
BIGF = 1.0e6
TWO_PI = 2.0 * np.pi
L519 = 519        # 629 - 110

_BF16 = ml_dtypes.bfloat16
_FP8 = ml_dtypes.float8_e4m3
_cache = {}
DBG_SKIP_SYN = False
DBG_SKIP_YIN = False
DBG_OMIT = set()
SYN_BUFS = 4
JD_SPLIT = 4


def _host_consts():
    j = np.arange(1280)
    k = np.arange(NBINS)
    w = np.zeros((1280, 2 * NBINS), dtype=np.float64)
    ang = 2.0 * np.pi * np.outer(j[:FRAME], k) / NFFT
    w[:FRAME, :NBINS] = np.cos(ang)
    w[:FRAME, NBINS:] = np.sin(ang)
    wdft = w.astype(_BF16)

    tau = np.arange(630)
    # 1/NFFT is folded into P (power spectrum) on-device so fp8 V stays O(1)
    v = np.cos(2.0 * np.pi * np.outer(k, tau) / NFFT)
    v[1:, :] *= 2.0
    vidft = v.astype(_BF16)
    vny = ((-1.0) ** tau).reshape(1, 630).astype(_BF16)

    alts = np.zeros((1280, 1), dtype=np.float64)
    alts[:FRAME, 0] = (-1.0) ** j[:FRAME]
    altsign = alts.astype(_BF16)

    ident = np.eye(128, dtype=_BF16)
    lt = (np.arange(128)[:, None] < np.arange(128)[None, :]).astype(np.float32)
    ones_row = np.ones((1, 128), dtype=np.float32)

    msel = []
    for h in (0, 1):
        m = np.zeros((128, 128), dtype=np.float32)
        m[h * 64 + np.arange(128) // 2, np.arange(128)] = 1.0
        msel.append(m)
    msel0 = np.zeros((128, 128), dtype=np.float32)
    msel0[np.arange(128) // 2, np.arange(128)] = 1.0

    harm_rev = np.arange(NH, 0, -1).astype(np.float32).reshape(1, NH)
    taus = np.arange(1, 630).astype(np.float32).reshape(1, 629)
    iota519 = np.arange(L519).astype(np.float32).reshape(1, L519)
    return dict(wdft=wdft, vidft=vidft, vny=vny, altsign=altsign, ident=ident,
                lt=lt, ones_row=ones_row, msel=msel, msel0=msel0,
                harm_rev=harm_rev, taus=taus, iota519=iota519)


def _ap(t, off_delta, free_dims):
    import concourse.bass as bass
    return bass.AP(t.tensor, t.offset + off_delta, [t.ap[0]] + free_dims)


def _build_nc():
    import concourse.bass as bass
    import concourse.bacc as bacc
    import concourse.mybir as mybir
    import concourse.tile as tile

    f32 = mybir.dt.float32
    bf16 = mybir.dt.bfloat16
    fp8 = mybir.dt.float8e4
    i32 = mybir.dt.int32
    AX = mybir.AxisListType.X
    OP = mybir.AluOpType
    ACTF = mybir.ActivationFunctionType

    nc = bacc.Bacc(trn_type="TRN2")

    audio = nc.dram_tensor("audio", [T], f32, kind="ExternalInput")
    pm_d = nc.dram_tensor("pm", [P, Q], f32, kind="ExternalInput")
    pmc_d = nc.dram_tensor("pmc", [P, Q], f32, kind="ExternalInput")
    msel_d = nc.dram_tensor("msel", [128, 128], f32, kind="ExternalInput")
    msel0_d = nc.dram_tensor("msel0", [128, 128], f32, kind="ExternalInput")
    wdft_d = nc.dram_tensor("wdft", [1280, 2 * NBINS], bf16, kind="ExternalInput")
    vidft_d = nc.dram_tensor("vidft", [NBINS, 630], bf16, kind="ExternalInput")
    vny_d = nc.dram_tensor("vny", [1, 630], bf16, kind="ExternalInput")
    alts_d = nc.dram_tensor("altsign", [1280, 1], bf16, kind="ExternalInput")
    ident_d = nc.dram_tensor("ident", [128, 128], bf16, kind="ExternalInput")
    lt_d = nc.dram_tensor("ltmask", [128, 128], f32, kind="ExternalInput")
    ones_d = nc.dram_tensor("ones_row", [1, 128], f32, kind="ExternalInput")
    harm_d = nc.dram_tensor("harm_rev", [1, NH], f32, kind="ExternalInput")
    ampsrev_d = nc.dram_tensor("amps_rev", [1, NH], f32, kind="ExternalInput")
    ratio_d = nc.dram_tensor("ratio_in", [1, 1], f32, kind="ExternalInput")
    taus_d = nc.dram_tensor("taus", [1, 629], f32, kind="ExternalInput")
    iota_d = nc.dram_tensor("iota519", [1, L519], f32, kind="ExternalInput")
    out_d = nc.dram_tensor("sig_out", [HALF], f32, kind="ExternalOutput")

    def bc(dram, n, parts=128):
        # partition-broadcast read of a [1, n] / [n] DRAM tensor
        return bass.AP(dram, 0, [[0, parts], [1, n]])

    with ExitStack() as ctx:
        tc = ctx.enter_context(tile.TileContext(nc))
        const = ctx.enter_context(tc.tile_pool(name="const", bufs=1))
        syn_keep = ctx.enter_context(tc.tile_pool(name="syn_keep", bufs=1))

        # ---- small constants ----
        harm_t = const.tile([128, NH], f32)
        nc.sync.dma_start(out=harm_t, in_=bc(harm_d, NH))
        ampr_raw = const.tile([128, NH], f32)
        nc.sync.dma_start(out=ampr_raw, in_=bc(ampsrev_d, NH))
        ratio_t = const.tile([128, 1], f32)
        nc.sync.dma_start(out=ratio_t, in_=bc(ratio_d, 1))
        amp_t = const.tile([128, NH], f32)
        nc.vector.tensor_scalar_mul(amp_t, ampr_raw, ratio_t[:, 0:1])
        # telescoping ratios in reversed layout: slot t (h=150-t) carries
        # amp_rev[t]/amp_rev[t+1]; last slot carries amp_1 itself. The scan's
        # running product over data1 then rebuilds amp_h exactly where the
        # mask is 1.
        ramp_t = const.tile([128, NH], f32)
        rec_amp = const.tile([128, NH], f32)
        nc.vector.reciprocal(rec_amp, amp_t)
        nc.vector.tensor_mul(ramp_t[:, 0:NH - 1], amp_t[:, 0:NH - 1],
                             rec_amp[:, 1:NH])
        nc.vector.tensor_copy(ramp_t[:, NH - 1:NH], amp_t[:, NH - 1:NH])
        taus_t = const.tile([128, 629], f32)
        nc.sync.dma_start(out=taus_t, in_=bc(taus_d, 629))
        iota_t = const.tile([128, L519], f32)
        nc.sync.dma_start(out=iota_t, in_=bc(iota_d, L519))
        msel_t = const.tile([128, 128], f32)
        nc.sync.dma_start(out=msel_t, in_=msel_d.ap())
        msel0_t = const.tile([128, 128], f32)
        nc.sync.dma_start(out=msel0_t, in_=msel0_d.ap())
        lt_t = const.tile([128, 128], f32)
        nc.sync.dma_start(out=lt_t, in_=lt_d.ap())
        ones_t = const.tile([1, 128], f32)
        nc.sync.dma_start(out=ones_t, in_=ones_d.ap())
        ident_t = const.tile([128, 128], bf16)
        nc.sync.dma_start(out=ident_t, in_=ident_d.ap())
        ident8_t = const.tile([128, 128], fp8)
        nc.vector.tensor_copy(ident8_t, ident_t)
        vny_t = const.tile([1, 630], bf16)
        nc.sync.dma_start(out=vny_t, in_=vny_d.ap())
        alts_t = const.tile([128, 10], bf16)
        nc.sync.dma_start(out=alts_t, in_=bass.AP(alts_d, 0, [[1, 128], [128, 10]]))
        pm_t = syn_keep.tile([P, Q], f32)
        nc.sync.dma_start(out=pm_t, in_=pm_d.ap())
        pmc_t = const.tile([P, Q], f32)
        nc.sync.dma_start(out=pmc_t, in_=pmc_d.ap())

        pitchS = const.tile([128, 1], f32)   # pitch * 2pi/FS per frame
        phi_t = syn_keep.tile([P, Q], f32)   # cumulative phase
        c_t = syn_keep.tile([P, Q], f32)     # mask cutoff pi/theta

        # ================= YIN =================
        if DBG_SKIP_YIN:
            nc.vector.memset(pitchS, 0.0)
        if True and not DBG_SKIP_YIN:
          with ExitStack() as yctx:
             ypool = yctx.enter_context(tc.tile_pool(name="yin", bufs=1))
             psT = yctx.enter_context(tc.tile_pool(name="psT", bufs=2, space="PSUM"))

             f_t = ypool.tile([128, FRAME], f32)
             nc.sync.dma_start(out=f_t, in_=bass.AP(audio, 0, [[1021, 128], [1, FRAME]]))
             fb = ypool.tile([128, 1280], bf16)
             nc.vector.memset(_ap(fb, FRAME, [[1, 1280 - FRAME]]), 0.0)
             nc.vector.tensor_copy(fb[:, 0:FRAME], f_t)

             # F^T chunks via PE transpose
             ftb_all = ypool.tile([128, 1280], bf16)
             ftb = [ftb_all[:, 128 * c:128 * (c + 1)] for c in range(10)]
             for c in range(10):
                 tp = psT.tile([128, 128], bf16, tag="tp")
                 nc.tensor.transpose(tp, fb[:, 128 * c:128 * (c + 1)], ident_t)
                 nc.vector.tensor_copy(ftb[c], tp)

             wt_all = ypool.tile([128, 10 * 2 * NBINS], bf16)
             wt = [wt_all[:, 2 * NBINS * c:2 * NBINS * (c + 1)] for c in range(10)]
             for c in range(10):
                 nc.sync.dma_start(out=wt[c], in_=wdft_d.ap()[128 * c:128 * (c + 1), :])

             with ExitStack() as sctx:
                 psS = sctx.enter_context(tc.tile_pool(name="psS", bufs=1, space="PSUM"))
                 psNy = sctx.enter_context(tc.tile_pool(name="psNy", bufs=1, space="PSUM"))
                 s_ps = psS.tile([128, 2 * NBINS], f32)
                 for kc in range(4):
                     for c in range(10):
                         nc.tensor.matmul(s_ps[:, 512 * kc:512 * (kc + 1)],
                                          lhsT=ftb[c], rhs=wt[c][:, 512 * kc:512 * (kc + 1)],
                                          start=(c == 0), stop=(c == 9))
                 sny_ps = psNy.tile([1, 128], f32)
                 for c in range(10):
                     nc.tensor.matmul(sny_ps, lhsT=alts_t[:, c:c + 1],
                                      rhs=ftb[c], start=(c == 0), stop=(c == 9))

                 sq_scale = float(1.0 / np.sqrt(NFFT))
                 t1 = ypool.tile([128, NBINS], f32)
                 nc.scalar.activation(t1, s_ps[:, 0:NBINS], ACTF.Square, scale=sq_scale)
                 t2 = ypool.tile([128, NBINS], f32)
                 nc.scalar.activation(t2, s_ps[:, NBINS:2 * NBINS], ACTF.Square, scale=sq_scale)
                 pb = ypool.tile([128, NBINS], bf16)
                 nc.vector.tensor_add(pb, t1, t2)
                 pnyT = ypool.tile([1, 128], bf16)
                 nc.scalar.activation(pnyT, sny_ps, ACTF.Square, scale=sq_scale)

             # transpose P and IDFT matmul -> corr
             ptb = ypool.tile([128, NBINS], bf16)
             for c in range(8):
                 tp = psT.tile([128, 128], bf16, tag="tp")
                 nc.tensor.transpose(tp, pb[:, 128 * c:128 * (c + 1)], ident_t)
                 nc.vector.tensor_copy(ptb[:, 128 * c:128 * (c + 1)], tp)

             vt_all = ypool.tile([128, 8 * 630], bf16)
             vt = [vt_all[:, 630 * c:630 * (c + 1)] for c in range(8)]
             for c in range(8):
                 nc.sync.dma_start(out=vt[c], in_=vidft_d.ap()[128 * c:128 * (c + 1), :])

             with ExitStack() as cctx:
                 psC = cctx.enter_context(tc.tile_pool(name="psC", bufs=1, space="PSUM"))
                 corr_ps = psC.tile([128, 1024], f32)
                 for (a, b) in ((0, 512), (512, 630)):
                     for c in range(8):
                         nc.tensor.matmul(corr_ps[:, a:b],
                                          lhsT=ptb[:, 128 * c:128 * (c + 1)],
                                          rhs=vt[c][:, a:b], start=(c == 0), stop=False)
                     nc.tensor.matmul(corr_ps[:, a:b], lhsT=pnyT,
                                      rhs=vny_t[:, a:b], start=False, stop=True)
                 corr_t = ypool.tile([128, 630], f32)
                 nc.vector.tensor_copy(corr_t, corr_ps[:, 0:630])

             # E = inclusive cumsum of F^2
             f2 = ypool.tile([128, FRAME], f32)
             nc.scalar.square(f2, f_t)
             e_t = ypool.tile([128, FRAME], f32)
             nc.vector.tensor_tensor_scan(e_t, f2, f2, 0.0, OP.add, OP.bypass)

             # d[tau] for tau=1..629 (dk)
             e_rev = _ap(e_t, 1258, [[-1, 629]])
             e_lo = _ap(e_t, 0, [[1, 629]])
             d_t = ypool.tile([128, 629], f32)
             nc.vector.tensor_sub(d_t, e_rev, e_lo)
             nc.vector.scalar_tensor_tensor(d_t, corr_t[:, 1:630], -2.0, d_t,
                                            OP.mult, OP.add)
             nc.vector.tensor_scalar_add(d_t, d_t, e_t[:, 1259:1260])

             # CMNDF decisions via cross-multiplication (denominators are
             # positive after the max clamp, so n/d < t  <=>  n < t*d and
             # n1/d1 >= n0/d0  <=>  n1*d0 >= n0*d1 - avoids the reciprocal)
             dsum = ypool.tile([128, 629], f32)
             nc.vector.tensor_tensor_scan(dsum, d_t, d_t, 0.0, OP.add, OP.bypass)
             nc.vector.tensor_scalar_max(dsum, dsum, 1e-5)
             numer = ypool.tile([128, 629], f32)
             nc.vector.tensor_mul(numer, d_t, taus_t)   # dk * tau
             sden = ypool.tile([128, 629], f32)
             nc.vector.tensor_scalar_mul(sden, dsum, 0.1)
             ns = numer[:, TAU_MIN:629]
             ds_den = dsum[:, TAU_MIN:629]

             # first_below
             below = ypool.tile([128, L519], f32)
             nc.vector.tensor_tensor(below, ns, sden[:, TAU_MIN:629], OP.is_lt)
             cand = ypool.tile([128, L519], f32)
             nc.vector.scalar_tensor_tensor(cand, below, -BIGF, iota_t, OP.mult, OP.add)
             mi = ypool.tile([128, 1], f32)
             nc.vector.tensor_reduce(mi, cand, AX, OP.min)
             fbv = ypool.tile([128, 1], f32)
             nc.vector.tensor_scalar_add(fbv, mi, BIGF)
             m1 = ypool.tile([128, 1], f32)
             nc.vector.tensor_scalar(m1, fbv, 1.0, None, OP.is_ge)
             m2 = ypool.tile([128, 1], f32)
             nc.vector.tensor_scalar(m2, fbv, 630.0, None, OP.is_le)
             nc.vector.tensor_mul(m1, m1, m2)
             fb_t = ypool.tile([128, 1], f32)
             nc.vector.scalar_tensor_tensor(fb_t, fbv, -630.0, m1, OP.add, OP.mult)
             nc.vector.tensor_scalar_add(fb_t, fb_t, 630.0)

             beyond = ypool.tile([128, L519], f32)
             nc.vector.tensor_scalar(beyond, iota_t, fb_t[:, 0:1], None, OP.is_ge)

             slope = ypool.tile([128, L519], f32)
             nc.vector.memset(slope, 1.0)
             xm1 = ypool.tile([128, L519 - 1], f32)
             nc.vector.tensor_mul(xm1, ns[:, 1:L519], ds_den[:, 0:L519 - 1])
             xm0 = ypool.tile([128, L519 - 1], f32)
             nc.vector.tensor_mul(xm0, ns[:, 0:L519 - 1], ds_den[:, 1:L519])
             nc.vector.tensor_tensor(slope[:, 0:L519 - 1], xm1, xm0, OP.is_ge)

             nc.vector.tensor_mul(beyond, beyond, slope)
             nc.vector.scalar_tensor_tensor(cand, beyond, -BIGF, iota_t, OP.mult, OP.add)
             nc.vector.tensor_reduce(mi, cand, AX, OP.min)
             tauv = ypool.tile([128, 1], f32)
             nc.vector.tensor_scalar_add(tauv, mi, BIGF)
             m3 = ypool.tile([128, 1], f32)
             nc.vector.tensor_scalar(m3, tauv, 630.0, None, OP.is_le)
             nc.vector.tensor_mul(tauv, tauv, m3)   # tau (0 if none)
             m4 = ypool.tile([128, 1], f32)
             nc.vector.tensor_scalar(m4, tauv, 1.0, None, OP.is_ge)
             ptau = ypool.tile([128, 1], f32)
             nc.vector.tensor_scalar_add(ptau, tauv, float(TAU_MIN + 1))
             rp = ypool.tile([128, 1], f32)
             nc.vector.reciprocal(rp, ptau)
             nc.vector.tensor_mul(pitchS, rp, m4)   # pitch/FS per frame (turns)

        # ============ phase & cutoff ============
        with ExitStack() as pctx:
            ppool = pctx.enter_context(tc.tile_pool(name="ph", bufs=1))
            psSm = pctx.enter_context(tc.tile_pool(name="psSm", bufs=1, space="PSUM"))

            pp_ps = psSm.tile([128, 1], f32)
            nc.tensor.matmul(pp_ps, lhsT=msel_t, rhs=pitchS, start=True, stop=True)
            ppartS = ppool.tile([128, 1], f32)
            nc.vector.tensor_copy(ppartS, pp_ps)

            p0_ps = psSm.tile([128, 1], f32)
            nc.tensor.matmul(p0_ps, lhsT=msel0_t, rhs=pitchS, start=True, stop=True)
            p0S = ppool.tile([128, 1], f32)
            nc.vector.tensor_copy(p0S, p0_ps)

            pmsum = ppool.tile([128, 1], f32)
            nc.vector.reduce_sum(pmsum, pmc_t, axis=AX)
            car_ps = psSm.tile([1, 1], f32)
            nc.tensor.matmul(car_ps, lhsT=p0S, rhs=pmsum, start=True, stop=True)
            car_sb = ppool.tile([1, 1], f32)
            nc.vector.tensor_copy(car_sb, car_ps)

            theta = ppool.tile([P, Q], f32)
            nc.vector.tensor_scalar_mul(theta, pm_t, ppartS[:, 0:1])
            sc_t = ppool.tile([P, Q], f32)
            nc.vector.tensor_tensor_scan(sc_t, theta, theta, 0.0, OP.add, OP.bypass)

            offs_ps = psSm.tile([128, 1], f32)
            nc.tensor.matmul(offs_ps, lhsT=lt_t, rhs=sc_t[:, Q - 1:Q],
                             start=True, stop=False)
            nc.tensor.matmul(offs_ps, lhsT=ones_t, rhs=car_sb,
                             start=False, stop=True)
            offs = ppool.tile([128, 1], f32)
            nc.vector.tensor_copy(offs, offs_ps)
            nc.vector.tensor_scalar_add(phi_t, sc_t, offs[:, 0:1])
            # reduce phi into [-0.5, 0.5] turns: phi -= round(phi). Harmonic
            # arguments y = phi*h then stay within +-75 turns, and the
            # per-element round() below recovers sin's [-pi, pi] domain
            # (the ACT Sin spline does no argument reduction in hardware).
            nphi = ppool.tile([P, Q], i32)
            nc.scalar.copy(nphi, phi_t)
            nc.vector.scalar_tensor_tensor(phi_t, nphi, -1.0, phi_t,
                                           OP.mult, OP.add)

            nc.vector.reciprocal(c_t, theta)
            nc.vector.tensor_scalar_mul(c_t, c_t, 0.5)

        # ============ synthesis ============
        spool = ctx.enter_context(tc.tile_pool(name="syn", bufs=SYN_BUFS))
        scpool = ctx.enter_context(tc.tile_pool(name="sc", bufs=2))
        sig = syn_keep.tile([P, Q], f32)
        FD = JC * SEG
        if DBG_SKIP_SYN:
            nc.vector.memset(sig, 0.0)
        JD = JD_SPLIT  # q's of the r-multiply on DVE; rest on GPSIMD
        # 4-stage software pipeline: s1 = phases/cmp/rmult, s2 = round-cast,
        # s3 = frac+sin, s4 = scan+extract. Each engine's per-iteration ops
        # only depend on >=1-iteration-old results, so no cross-engine
        # bubbles (ACT's cast fills the window while DVE runs frac).
        st = {}
        lo = 0 if not DBG_SKIP_SYN else NCHUNK
        for ch in range(lo, NCHUNK + 3):
            if ch < NCHUNK:
                q0 = ch * JC
                A = spool.tile([128, FD], f32, tag="A")
                Cm = spool.tile([128, FD], f32, tag="Cm")
                slotsA = _ap(A, 1, [[SEG, JC], [1, NH]])
                padsA = _ap(A, 0, [[SEG, JC]])
                slotsCm = _ap(Cm, 1, [[SEG, JC], [1, NH]])
                padsCm = _ap(Cm, 0, [[SEG, JC]])
                phi_rep = _ap(phi_t, q0, [[1, JC], [0, NH]])
                c_rep = _ap(c_t, q0, [[1, JC], [0, NH]])
                harm_rep = _ap(harm_t, 0, [[0, JC], [1, NH]])
                nc.gpsimd.memset(padsA, 0.0)
                nc.gpsimd.tensor_tensor(slotsA, phi_rep, harm_rep, OP.mult)
                nc.vector.memset(padsCm, 0.0)
                nc.vector.tensor_tensor(slotsCm, c_rep, harm_rep, OP.is_gt)
                if JD > 0:
                    sd = _ap(Cm, 1, [[SEG, JD], [1, NH]])
                    nc.vector.tensor_tensor(sd, sd, _ap(ramp_t, 0, [[0, JD], [1, NH]]), OP.mult)
                if JD < JC:
                    sg = _ap(Cm, 1 + SEG * JD, [[SEG, JC - JD], [1, NH]])
                    nc.gpsimd.tensor_tensor(sg, sg, _ap(ramp_t, 0, [[0, JC - JD], [1, NH]]), OP.mult)
                st[ch] = [A, Cm, None, None]
            if ch - 1 >= lo and ch - 1 < NCHUNK:
                A1, _, _, _ = st[ch - 1]
                N = spool.tile([128, FD], i32, tag="N")
                nc.scalar.copy(_ap(N, 1, [[SEG, JC], [1, NH]]),
                               _ap(A1, 1, [[SEG, JC], [1, NH]]))
                st[ch - 1][2] = N
            if ch - 2 >= lo and ch - 2 < NCHUNK:
                A2, _, N2, _ = st[ch - 2]
                sl = _ap(A2, 1, [[SEG, JC], [1, NH]])
                nc.vector.scalar_tensor_tensor(sl, _ap(N2, 1, [[SEG, JC], [1, NH]]),
                                               -1.0, sl, OP.mult, OP.add)
                nc.scalar.activation(sl, sl, ACTF.Sin, scale=float(TWO_PI))
            if ch - 3 >= lo and ch - 3 < NCHUNK:
                A3, Cm3, _, _ = st.pop(ch - 3)
                p0 = (ch - 3) * JC
                Sc = scpool.tile([128, FD], f32, tag="Sc")
                nc.vector.tensor_tensor_scan(Sc, A3, Cm3, 0.0, OP.add, OP.mult)
                nc.scalar.copy(sig[:, p0:p0 + JC], _ap(Sc, NH, [[SEG, JC]]))

        nc.sync.dma_start(out=bass.AP(out_d, 0, [[Q, P], [1, Q]]), in_=sig)

    nc.finalize()
    return nc


def kernel(audio, pitch_mult, amplitudes, ratio):
    from concourse.bass_utils import run_bass_kernel_spmd

    audio = np.ascontiguousarray(np.asarray(audio, dtype=np.float32))
    pitch_mult = np.ascontiguousarray(np.asarray(pitch_mult, dtype=np.float32))
    amplitudes = np.ascontiguousarray(np.asarray(amplitudes, dtype=np.float32))
    ratio = np.ascontiguousarray(np.asarray(ratio, dtype=np.float32))

    if "nc" not in _cache:
        _cache["nc"] = _build_nc()
        _cache["consts"] = _host_consts()
    nc = _cache["nc"]
    cc = _cache["consts"]

    amps_rev = amplitudes[::-1].reshape(1, NH).copy()
    in_maps = []
    for core in range(8):
        r, h = core // 2, core % 2
        pm = pitch_mult[r, h * HALF:(h + 1) * HALF].reshape(P, Q).copy()
        if h == 1:
            pmc = pitch_mult[r, 0:HALF].reshape(P, Q).copy()
        else:
            pmc = np.zeros((P, Q), dtype=np.float32)
        in_maps.append({
            "audio": audio[r].copy(),
            "pm": pm,
            "pmc": pmc,
            "msel": cc["msel"][h],
            "msel0": cc["msel0"],
            "wdft": cc["wdft"],
            "vidft": cc["vidft"],
            "vny": cc["vny"],
            "altsign": cc["altsign"],
            "ident": cc["ident"],
            "ltmask": cc["lt"],
            "ones_row": cc["ones_row"],
            "harm_rev": cc["harm_rev"],
            "amps_rev": amps_rev,
            "ratio_in": ratio.reshape(1, 1),
            "taus": cc["taus"],
            "iota519": cc["iota519"],
        })

    res = run_bass_kernel_spmd(nc, in_maps, core_ids=list(range(8)))
    out = np.zeros((B, T), dtype=np.float32)
    for core in range(8):
        r, h = core // 2, core % 2
        out[r, h * HALF:(h + 1) * HALF] = res.results[core]["sig_out"]
    return out



# revision 7
# speedup vs baseline: 1.6860x; 1.6860x over previous
"""Trainium2 Bass kernel for nn_ExcitationModule (YIN pitch -> harmonic synthesis).

Sharding: B=4 rows x 2 halves of T=131072 across 8 cores (pure data parallel;
the phase cumsum carry for the second half of each row is recomputed locally
from the first-half pitch_mult, so no collectives are needed).

Per core layout: 65536 samples as [128 partitions x 512], t = p*512 + q.
Pipeline per core:
  1. YIN on the full row (128 frames on partitions): autocorrelation via a
     2048-point DFT as bf16 PE matmuls (exact linear autocorr since
     1260+630 <= 2048), difference function, CMNDF, threshold/argmax logic.
  2. phase = cumsum(2*pi*f0/FS) via per-partition scan + PE lower-triangular
     prefix matmul + carry; phi reduced to [-0.5, 0.5] turns.
  3. signal = sum_h amp_h*mask*sin(h*phi): per chunk of 16 q-columns,
     A = phi*h (Pool/DVE f32), N = round(A) (ACT int16 copy -- the scalar
     engine's convert rounds), W = A - N in fp16 (DVE), per-column Nyquist
     masks for h>=57 only (h<=56 is never masked since f0 < 394 Hz) via
     4x-mode tensor_scalar is_lt against c = FS/(2 f0), masked W -> ACT Sin,
     then the harmonic reduction runs on the otherwise-idle PE as 150
     PSUM-accumulating matmuls whose stationary weights are amp_h * I
     (fp16 diagonal matrices built host-side from amplitudes*ratio).
     The reference's +1e-7 mask epsilon term is dropped (contributes <~1e-6).
     Zero-pitch samples stay exactly 0 end to end: phi=0 -> W=0 -> sin(0)=0.
"""

import numpy as np
import ml_dtypes
from contextlib import ExitStack

FS = 44100.0
NH = 150
TAU_MIN = 110
FRAME = 1260
B, T = 4, 131072
NF = 128          # frames per row (T//1024)
NFFT = 2048
NBINS = 1024      # DFT bins handled by the main matmul; Nyquist separate
HALF = 65536      # samples per core
P, Q = 128, 512   # per-core layout [P partitions, Q]
JC = 16           # q columns per synthesis chunk
NCHUNK = Q // JC  # 32
FD = JC * NH      # chunk free size (no pad slots)
HM0 = 56          # harmonics 1..56 are never Nyquist-masked (f0 < 394 Hz)
NMASK = NH - HM0  # 94 maskable harmonics per column
BIGF = 1.0e6
TWO_PI = 2.0 * np.pi
L519 = 519        # 629 - 110
DVE_A_CHUNKS = 3  # chunks whose A=phi*h runs on DVE instead of Pool

_BF16 = ml_dtypes.bfloat16
_cache = {}


def _host_consts():
    j = np.arange(1280)
    k = np.arange(NBINS)
    w = np.zeros((1280, 2 * NBINS), dtype=np.float64)
    ang = 2.0 * np.pi * np.outer(j[:FRAME], k) / NFFT
    w[:FRAME, :NBINS] = np.cos(ang)
    w[:FRAME, NBINS:] = np.sin(ang)
    wdft = w.astype(_BF16)

    tau = np.arange(630)
    # 1/NFFT is folded into P (power spectrum) on-device so bf16 V stays O(1)
    v = np.cos(2.0 * np.pi * np.outer(k, tau) / NFFT)
    v[1:, :] *= 2.0
    vidft = v.astype(_BF16)
    vny = ((-1.0) ** tau).reshape(1, 630).astype(_BF16)

    alts = np.zeros((1280, 1), dtype=np.float64)
    alts[:FRAME, 0] = (-1.0) ** j[:FRAME]
    altsign = alts.astype(_BF16)

    ident = np.eye(128, dtype=_BF16)
    lt = (np.arange(128)[:, None] < np.arange(128)[None, :]).astype(np.float32)
    ones_row = np.ones((1, 128), dtype=np.float32)

    msel = []
    for h in (0, 1):
        m = np.zeros((128, 128), dtype=np.float32)
        m[h * 64 + np.arange(128) // 2, np.arange(128)] = 1.0
        msel.append(m)
    msel0 = np.zeros((128, 128), dtype=np.float32)
    msel0[np.arange(128) // 2, np.arange(128)] = 1.0

    harm_fwd = np.arange(1, NH + 1).astype(np.float32).reshape(1, NH)
    harmmat = np.repeat(np.arange(HM0 + 1, NH + 1), JC).astype(np.float32).reshape(1, NMASK * JC)
    taus = np.arange(1, 630).astype(np.float32).reshape(1, 629)
    iota519 = np.arange(L519).astype(np.float32).reshape(1, L519)
    return dict(wdft=wdft, vidft=vidft, vny=vny, altsign=altsign, ident=ident,
                lt=lt, ones_row=ones_row, msel=msel, msel0=msel0,
                harm_fwd=harm_fwd, harmmat=harmmat, taus=taus, iota519=iota519)


def _ap(t, off_delta, free_dims):
    import concourse.bass as bass
    return bass.AP(t.tensor, t.offset + off_delta, [t.ap[0]] + free_dims)


def _build_nc():
    import concourse.bass as bass
    import concourse.bacc as bacc
    import concourse.mybir as mybir
    import concourse.tile as tile

    f32 = mybir.dt.float32
    bf16 = mybir.dt.bfloat16
    fp16 = mybir.dt.float16
    i16 = mybir.dt.int16
    AX = mybir.AxisListType.X
    OP = mybir.AluOpType
    ACTF = mybir.ActivationFunctionType

    nc = bacc.Bacc(trn_type="TRN2")

    audio = nc.dram_tensor("audio", [T], f32, kind="ExternalInput")
    pm_d = nc.dram_tensor("pm", [P, Q], f32, kind="ExternalInput")
    pmc_d = nc.dram_tensor("pmc", [P, Q], f32, kind="ExternalInput")
    msel_d = nc.dram_tensor("msel", [128, 128], f32, kind="ExternalInput")
    msel0_d = nc.dram_tensor("msel0", [128, 128], f32, kind="ExternalInput")
    wdft_d = nc.dram_tensor("wdft", [1280, 2 * NBINS], bf16, kind="ExternalInput")
    vidft_d = nc.dram_tensor("vidft", [NBINS, 630], bf16, kind="ExternalInput")
    vny_d = nc.dram_tensor("vny", [1, 630], bf16, kind="ExternalInput")
    alts_d = nc.dram_tensor("altsign", [1280, 1], bf16, kind="ExternalInput")
    ident_d = nc.dram_tensor("ident", [128, 128], bf16, kind="ExternalInput")
    lt_d = nc.dram_tensor("ltmask", [128, 128], f32, kind="ExternalInput")
    ones_d = nc.dram_tensor("ones_row", [1, 128], f32, kind="ExternalInput")
    harm_d = nc.dram_tensor("harm_fwd", [1, NH], f32, kind="ExternalInput")
    harmm_d = nc.dram_tensor("harmmat", [1, NMASK * JC], f32, kind="ExternalInput")
    diag_d = nc.dram_tensor("diagm", [128, NH * 128], fp16, kind="ExternalInput")
    taus_d = nc.dram_tensor("taus", [1, 629], f32, kind="ExternalInput")
    iota_d = nc.dram_tensor("iota519", [1, L519], f32, kind="ExternalInput")
    out_d = nc.dram_tensor("sig_out", [HALF], f32, kind="ExternalOutput")

    def bc(dram, n, parts=128):
        # partition-broadcast read of a [1, n] / [n] DRAM tensor
        return bass.AP(dram, 0, [[0, parts], [1, n]])

    with ExitStack() as ctx:
        tc = ctx.enter_context(tile.TileContext(nc))
        const = ctx.enter_context(tc.tile_pool(name="const", bufs=1))
        syn_keep = ctx.enter_context(tc.tile_pool(name="syn_keep", bufs=1))

        # ---- constants (audio + DFT weights queue first; diag queues last,
        # it is only needed once synthesis starts) ----
        yin_f = const.tile([128, FRAME], f32)
        nc.sync.dma_start(out=yin_f, in_=bass.AP(audio, 0, [[1021, 128], [1, FRAME]]))
        harm_t = const.tile([128, NH], f32)
        harmm_f = const.tile([128, NMASK * JC], f32)
        harmm_t = const.tile([128, NMASK * JC], fp16)
        taus_t = const.tile([128, 629], f32)
        iota_t = const.tile([128, L519], f32)
        msel_t = const.tile([128, 128], f32)
        msel0_t = const.tile([128, 128], f32)
        lt_t = const.tile([128, 128], f32)
        ones_t = const.tile([1, 128], f32)
        ident_t = const.tile([128, 128], bf16)
        nc.sync.dma_start(out=ident_t, in_=ident_d.ap())
        vny_t = const.tile([1, 630], bf16)
        alts_t = const.tile([128, 10], bf16)
        pm_t = syn_keep.tile([P, Q], f32)
        pmc_t = const.tile([P, Q], f32)
        diag_t = const.tile([128, NH * 128], fp16)

        pitchS = const.tile([128, 1], f32)   # pitch/FS per frame (turns)
        phi_t = syn_keep.tile([P, Q], f32)   # cumulative phase, frac-reduced
        c_t = syn_keep.tile([P, Q], f32)     # mask cutoff FS/(2 f0) = 0.5/theta

        # ================= YIN =================
        with ExitStack() as yctx:
            ypool = yctx.enter_context(tc.tile_pool(name="yin", bufs=1))
            psT = yctx.enter_context(tc.tile_pool(name="psT", bufs=2, space="PSUM"))

            f_t = yin_f
            fb = ypool.tile([128, 1280], bf16)
            nc.vector.memset(_ap(fb, FRAME, [[1, 1280 - FRAME]]), 0.0)
            nc.vector.tensor_copy(fb[:, 0:FRAME], f_t)

            # F^T chunks via PE transpose
            ftb_all = ypool.tile([128, 1280], bf16)
            ftb = [ftb_all[:, 128 * c:128 * (c + 1)] for c in range(10)]
            for c in range(10):
                tp = psT.tile([128, 128], bf16, tag="tp")
                nc.tensor.transpose(tp, fb[:, 128 * c:128 * (c + 1)], ident_t)
                nc.vector.tensor_copy(ftb[c], tp)

            wt_all = ypool.tile([128, 10 * 2 * NBINS], bf16)
            wt = [wt_all[:, 2 * NBINS * c:2 * NBINS * (c + 1)] for c in range(10)]
            for c in range(10):
                nc.sync.dma_start(out=wt[c], in_=wdft_d.ap()[128 * c:128 * (c + 1), :])
            # deferred const loads: queue behind the DFT weights so they do
            # not delay the DFT matmuls; all are consumed later than that.
            nc.sync.dma_start(out=vny_t, in_=vny_d.ap())
            nc.sync.dma_start(out=alts_t, in_=bass.AP(alts_d, 0, [[1, 128], [128, 10]]))
            nc.sync.dma_start(out=taus_t, in_=bc(taus_d, 629))
            nc.sync.dma_start(out=iota_t, in_=bc(iota_d, L519))
            nc.sync.dma_start(out=msel_t, in_=msel_d.ap())
            nc.sync.dma_start(out=msel0_t, in_=msel0_d.ap())
            nc.sync.dma_start(out=lt_t, in_=lt_d.ap())
            nc.sync.dma_start(out=ones_t, in_=ones_d.ap())
            nc.sync.dma_start(out=pm_t, in_=pm_d.ap())
            nc.sync.dma_start(out=pmc_t, in_=pmc_d.ap())
            nc.sync.dma_start(out=harm_t, in_=bc(harm_d, NH))
            nc.sync.dma_start(out=harmm_f, in_=bc(harmm_d, NMASK * JC))
            nc.vector.tensor_copy(harmm_t, harmm_f)

            with ExitStack() as sctx:
                psS = sctx.enter_context(tc.tile_pool(name="psS", bufs=1, space="PSUM"))
                psNy = sctx.enter_context(tc.tile_pool(name="psNy", bufs=1, space="PSUM"))
                s_ps = psS.tile([128, 2 * NBINS], f32)
                for c in range(10):
                    for kc in range(4):
                        nc.tensor.matmul(s_ps[:, 512 * kc:512 * (kc + 1)],
                                         lhsT=ftb[c], rhs=wt[c][:, 512 * kc:512 * (kc + 1)],
                                         start=(c == 0), stop=(c == 9))
                sny_ps = psNy.tile([1, 128], f32)
                for c in range(10):
                    nc.tensor.matmul(sny_ps, lhsT=alts_t[:, c:c + 1],
                                     rhs=ftb[c], start=(c == 0), stop=(c == 9))

                sq_scale = float(1.0 / np.sqrt(NFFT))
                t1 = ypool.tile([128, NBINS], f32)
                nc.scalar.activation(t1, s_ps[:, 0:NBINS], ACTF.Square, scale=sq_scale)
                t2 = ypool.tile([128, NBINS], f32)
                nc.scalar.activation(t2, s_ps[:, NBINS:2 * NBINS], ACTF.Square, scale=sq_scale)
                pb = ypool.tile([128, NBINS], bf16)
                nc.vector.tensor_add(pb, t1, t2)
                pnyT = ypool.tile([1, 128], bf16)
                nc.scalar.activation(pnyT, sny_ps, ACTF.Square, scale=sq_scale)

            # transpose P and IDFT matmul -> corr
            ptb = ypool.tile([128, NBINS], bf16)
            for c in range(8):
                tp = psT.tile([128, 128], bf16, tag="tp")
                nc.tensor.transpose(tp, pb[:, 128 * c:128 * (c + 1)], ident_t)
                nc.vector.tensor_copy(ptb[:, 128 * c:128 * (c + 1)], tp)

            vt_all = ypool.tile([128, 8 * 630], bf16)
            vt = [vt_all[:, 630 * c:630 * (c + 1)] for c in range(8)]
            for c in range(8):
                nc.sync.dma_start(out=vt[c], in_=vidft_d.ap()[128 * c:128 * (c + 1), :])
            nc.sync.dma_start(out=diag_t, in_=diag_d.ap())

            with ExitStack() as cctx:
                psC = cctx.enter_context(tc.tile_pool(name="psC", bufs=1, space="PSUM"))
                corr_ps = psC.tile([128, 1024], f32)
                for c in range(8):
                    for (a, b) in ((0, 512), (512, 630)):
                        nc.tensor.matmul(corr_ps[:, a:b],
                                         lhsT=ptb[:, 128 * c:128 * (c + 1)],
                                         rhs=vt[c][:, a:b], start=(c == 0), stop=False)
                for (a, b) in ((0, 512), (512, 630)):
                    nc.tensor.matmul(corr_ps[:, a:b], lhsT=pnyT,
                                     rhs=vny_t[:, a:b], start=False, stop=True)
                corr_t = ypool.tile([128, 630], f32)
                nc.vector.tensor_copy(corr_t, corr_ps[:, 0:630])

            # E = inclusive cumsum of F^2
            f2 = ypool.tile([128, FRAME], f32)
            nc.scalar.square(f2, f_t)
            e_t = ypool.tile([128, FRAME], f32)
            nc.vector.tensor_tensor_scan(e_t, f2, f2, 0.0, OP.add, OP.bypass)

            # d[tau] for tau=1..629 (dk)
            e_rev = _ap(e_t, 1258, [[-1, 629]])
            e_lo = _ap(e_t, 0, [[1, 629]])
            d_t = ypool.tile([128, 629], f32)
            nc.vector.tensor_sub(d_t, e_rev, e_lo)
            nc.vector.scalar_tensor_tensor(d_t, corr_t[:, 1:630], -2.0, d_t,
                                           OP.mult, OP.add)
            nc.vector.tensor_scalar_add(d_t, d_t, e_t[:, 1259:1260])

            # CMNDF decisions via cross-multiplication (denominators are
            # positive after the max clamp, so n/d < t  <=>  n < t*d and
            # n1/d1 >= n0/d0  <=>  n1*d0 >= n0*d1 - avoids the reciprocal)
            dsum = ypool.tile([128, 629], f32)
            nc.vector.tensor_tensor_scan(dsum, d_t, d_t, 0.0, OP.add, OP.bypass)
            nc.vector.tensor_scalar_max(dsum, dsum, 1e-5)
            numer = ypool.tile([128, 629], f32)
            nc.gpsimd.tensor_mul(numer, d_t, taus_t)   # dk * tau
            sden = ypool.tile([128, 629], f32)
            nc.vector.tensor_scalar_mul(sden, dsum, 0.1)
            ns = numer[:, TAU_MIN:629]
            ds_den = dsum[:, TAU_MIN:629]

            # first_below
            below = ypool.tile([128, L519], f32)
            nc.vector.tensor_tensor(below, ns, sden[:, TAU_MIN:629], OP.is_lt)
            cand = ypool.tile([128, L519], f32)
            nc.vector.scalar_tensor_tensor(cand, below, -BIGF, iota_t, OP.mult, OP.add)
            mi = ypool.tile([128, 1], f32)
            nc.vector.tensor_reduce(mi, cand, AX, OP.min)
            fbv = ypool.tile([128, 1], f32)
            nc.vector.tensor_scalar_add(fbv, mi, BIGF)
            m1 = ypool.tile([128, 1], f32)
            nc.vector.tensor_scalar(m1, fbv, 1.0, None, OP.is_ge)
            m2 = ypool.tile([128, 1], f32)
            nc.vector.tensor_scalar(m2, fbv, 630.0, None, OP.is_le)
            nc.vector.tensor_mul(m1, m1, m2)
            fb_t = ypool.tile([128, 1], f32)
            nc.vector.scalar_tensor_tensor(fb_t, fbv, -630.0, m1, OP.add, OP.mult)
            nc.vector.tensor_scalar_add(fb_t, fb_t, 630.0)

            beyond = ypool.tile([128, L519], f32)
            nc.vector.tensor_scalar(beyond, iota_t, fb_t[:, 0:1], None, OP.is_ge)

            slope = ypool.tile([128, L519], f32)
            nc.vector.memset(slope, 1.0)
            xm1 = ypool.tile([128, L519 - 1], f32)
            nc.gpsimd.tensor_mul(xm1, ns[:, 1:L519], ds_den[:, 0:L519 - 1])
            xm0 = ypool.tile([128, L519 - 1], f32)
            nc.gpsimd.tensor_mul(xm0, ns[:, 0:L519 - 1], ds_den[:, 1:L519])
            nc.vector.tensor_tensor(slope[:, 0:L519 - 1], xm1, xm0, OP.is_ge)

            nc.vector.tensor_mul(beyond, beyond, slope)
            nc.vector.scalar_tensor_tensor(cand, beyond, -BIGF, iota_t, OP.mult, OP.add)
            nc.vector.tensor_reduce(mi, cand, AX, OP.min)
            tauv = ypool.tile([128, 1], f32)
            nc.vector.tensor_scalar_add(tauv, mi, BIGF)
            m3 = ypool.tile([128, 1], f32)
            nc.vector.tensor_scalar(m3, tauv, 630.0, None, OP.is_le)
            nc.vector.tensor_mul(tauv, tauv, m3)   # tau (0 if none)
            m4 = ypool.tile([128, 1], f32)
            nc.vector.tensor_scalar(m4, tauv, 1.0, None, OP.is_ge)
            ptau = ypool.tile([128, 1], f32)
            nc.vector.tensor_scalar_add(ptau, tauv, float(TAU_MIN + 1))
            rp = ypool.tile([128, 1], f32)
            nc.vector.reciprocal(rp, ptau)
            nc.vector.tensor_mul(pitchS, rp, m4)   # pitch/FS per frame (turns)

        # ============ phase & cutoff ============
        with ExitStack() as pctx:
            ppool = pctx.enter_context(tc.tile_pool(name="ph", bufs=1))
            psSm = pctx.enter_context(tc.tile_pool(name="psSm", bufs=1, space="PSUM"))

            pp_ps = psSm.tile([128, 1], f32)
            nc.tensor.matmul(pp_ps, lhsT=msel_t, rhs=pitchS, start=True, stop=True)
            ppartS = ppool.tile([128, 1], f32)
            nc.vector.tensor_copy(ppartS, pp_ps)

            p0_ps = psSm.tile([128, 1], f32)
            nc.tensor.matmul(p0_ps, lhsT=msel0_t, rhs=pitchS, start=True, stop=True)
            p0S = ppool.tile([128, 1], f32)
            nc.vector.tensor_copy(p0S, p0_ps)

            pmsum = ppool.tile([128, 1], f32)
            nc.vector.reduce_sum(pmsum, pmc_t, axis=AX)
            car_ps = psSm.tile([1, 1], f32)
            nc.tensor.matmul(car_ps, lhsT=p0S, rhs=pmsum, start=True, stop=True)
            car_sb = ppool.tile([1, 1], f32)
            nc.vector.tensor_copy(car_sb, car_ps)

            theta = ppool.tile([P, Q], f32)
            nc.vector.tensor_scalar_mul(theta, pm_t, ppartS[:, 0:1])
            sc_t = ppool.tile([P, Q], f32)
            nc.vector.tensor_tensor_scan(sc_t, theta, theta, 0.0, OP.add, OP.bypass)

            offs_ps = psSm.tile([128, 1], f32)
            nc.tensor.matmul(offs_ps, lhsT=lt_t, rhs=sc_t[:, Q - 1:Q],
                             start=True, stop=False)
            nc.tensor.matmul(offs_ps, lhsT=ones_t, rhs=car_sb,
                             start=False, stop=True)
            offs = ppool.tile([128, 1], f32)
            nc.vector.tensor_copy(offs, offs_ps)
            nc.vector.tensor_scalar_add(phi_t, sc_t, offs[:, 0:1])
            # reduce phi into [-0.5, 0.5] turns: phi -= round(phi). The scalar
            # engine's fp32->int convert rounds to nearest, so per-chunk
            # A = phi*h stays within +-75.5 and A - round(A) lands in
            # [-0.5, 0.5] -- the ACT Sin domain after the 2*pi scale.
            nphi = ppool.tile([P, Q], i16)
            nc.scalar.copy(nphi, phi_t)
            nc.vector.scalar_tensor_tensor(phi_t, nphi, -1.0, phi_t,
                                           OP.mult, OP.add)

            nc.vector.reciprocal(c_t, theta)
            nc.vector.tensor_scalar_mul(c_t, c_t, 0.5)

        # ============ synthesis ============
        # 6-stage software pipeline: st0 A=phi*h (Pool/DVE) + mask (DVE),
        # st1 N=round(A) (ACT), st2 W=A-N (DVE/Pool), st3 sin (ACT, written
        # h-major), st4 mask apply (DVE), st5 PE diag-matmul reduce + copy.
        # Stages are emitted skewed so each in-order engine queue never has a
        # younger chunk's work stuck behind a wait on another engine.
        apool = ctx.enter_context(tc.tile_pool(name="synA", bufs=4))
        mpool = ctx.enter_context(tc.tile_pool(name="synM", bufs=6))
        npool = ctx.enter_context(tc.tile_pool(name="synN", bufs=3))
        wpool = ctx.enter_context(tc.tile_pool(name="synW", bufs=3))
        s2pool = ctx.enter_context(tc.tile_pool(name="synS", bufs=4))
        pacc = ctx.enter_context(tc.tile_pool(name="pacc", bufs=4, space="PSUM"))
        sig = syn_keep.tile([P, Q], f32)
        c16 = syn_keep.tile([P, Q], fp16)
        nc.vector.tensor_copy(c16, c_t)
        SUB0 = HM0 * JC
        NSUB = NMASK * JC
        st = {}
        def cls_of(ch):
            return 2 if ch in (5, 16, 27) else (ch % 2)
        for i in range(NCHUNK + 5):
            if i < NCHUNK:
                ch = i; q0 = ch * JC
                A = apool.tile([128, FD], f32, tag="A")
                slotsA = _ap(A, 0, [[NH, JC], [1, NH]])
                phi_rep = _ap(phi_t, q0, [[1, JC], [0, NH]])
                harm_rep = _ap(harm_t, 0, [[0, JC], [1, NH]])
                if cls_of(ch) in (0, 2):
                    nc.vector.tensor_tensor(slotsA, phi_rep, harm_rep, OP.mult)
                else:
                    nc.gpsimd.tensor_tensor(slotsA, phi_rep, harm_rep, OP.mult)
                Msk = mpool.tile([128, NSUB], fp16, tag="Msk")
                c_rep = _ap(c16, q0, [[0, NMASK], [1, JC]])
                nc.vector.tensor_tensor(Msk, c_rep, harmm_t, OP.is_gt)
                st[ch] = [A, Msk, None, None, None]
            ch = i - 1
            if 0 <= ch < NCHUNK:
                A = st[ch][0]
                N = npool.tile([128, FD], i16, tag="N")
                nc.scalar.copy(N, A)
                st[ch][2] = N
            ch = i - 2
            if 0 <= ch < NCHUNK:
                A, _, N, _, _ = st[ch]
                W = wpool.tile([128, FD], fp16, tag="W")
                if cls_of(ch) == 0:
                    nc.gpsimd.tensor_tensor(W, A, N, OP.subtract)
                else:
                    nc.vector.scalar_tensor_tensor(W, N, -1.0, A, OP.mult, OP.add)
                st[ch][3] = W
            ch = i - 3
            if 0 <= ch < NCHUNK:
                W = st[ch][3]
                S = s2pool.tile([128, FD], fp16, tag="S")
                S_hmaj = _ap(S, 0, [[1, JC], [JC, NH]])
                nc.scalar.activation(S_hmaj, _ap(W, 0, [[NH, JC], [1, NH]]),
                                     ACTF.Sin, scale=float(TWO_PI))
                st[ch][4] = S
            ch = i - 4
            if 0 <= ch < NCHUNK:
                S, Msk = st[ch][4], st[ch][1]
                nc.vector.tensor_tensor(S[:, SUB0:SUB0 + NSUB],
                                        S[:, SUB0:SUB0 + NSUB], Msk, OP.mult)
            ch = i - 5
            if 0 <= ch < NCHUNK:
                S = st[ch][4]; q0 = ch * JC
                acc = pacc.tile([128, JC], f32, tag="acc")
                for h in range(NH):
                    nc.tensor.matmul(acc,
                                     lhsT=diag_t[:, h * 128:(h + 1) * 128],
                                     rhs=S[:, h * JC:(h + 1) * JC],
                                     start=(h == 0), stop=(h == NH - 1))
                nc.vector.tensor_copy(sig[:, q0:q0 + JC], acc)
                del st[ch]

        nc.sync.dma_start(out=bass.AP(out_d, 0, [[Q, P], [1, Q]]), in_=sig)

    nc.finalize()
    return nc


def kernel(audio, pitch_mult, amplitudes, ratio):
    from concourse.bass_utils import run_bass_kernel_spmd

    audio = np.ascontiguousarray(np.asarray(audio, dtype=np.float32))
    pitch_mult = np.ascontiguousarray(np.asarray(pitch_mult, dtype=np.float32))
    amplitudes = np.ascontiguousarray(np.asarray(amplitudes, dtype=np.float32))
    ratio = np.ascontiguousarray(np.asarray(ratio, dtype=np.float32))

    if "nc" not in _cache:
        _cache["nc"] = _build_nc()
        _cache["consts"] = _host_consts()
    nc = _cache["nc"]
    cc = _cache["consts"]

    amp = (amplitudes * ratio[0]).astype(np.float32)
    diagm = np.zeros((128, NH * 128), dtype=np.float16)
    ar = np.arange(128)
    for h in range(NH):
        diagm[ar, h * 128 + ar] = amp[h]

    in_maps = []
    for core in range(8):
        r, h = core // 2, core % 2
        pm = pitch_mult[r, h * HALF:(h + 1) * HALF].reshape(P, Q).copy()
        if h == 1:
            pmc = pitch_mult[r, 0:HALF].reshape(P, Q).copy()
        else:
            pmc = np.zeros((P, Q), dtype=np.float32)
        in_maps.append({
            "audio": audio[r].copy(),
            "pm": pm,
            "pmc": pmc,
            "msel": cc["msel"][h],
            "msel0": cc["msel0"],
            "wdft": cc["wdft"],
            "vidft": cc["vidft"],
            "vny": cc["vny"],
            "altsign": cc["altsign"],
            "ident": cc["ident"],
            "ltmask": cc["lt"],
            "ones_row": cc["ones_row"],
            "harm_fwd": cc["harm_fwd"],
            "harmmat": cc["harmmat"],
            "diagm": diagm,
            "taus": cc["taus"],
            "iota519": cc["iota519"],
        })

    res = run_bass_kernel_spmd(nc, in_maps, core_ids=list(range(8)))
    out = np.zeros((B, T), dtype=np.float32)
    for core in range(8):
        r, h = core // 2, core % 2
        out[r, h * HALF:(h + 1) * HALF] = res.results[core]["sig_out"]
    return out
